# revision 15
# baseline (speedup 1.0000x reference)
"""Trainium2 Bass kernel: multi-head attention with 1x1-conv K/V projections,
per-head GhostBatchNorm (eval-mode affine), key+query masking, softmax.

Sharding: pure data parallelism over the batch axis (16 batches -> 8 cores,
2 per core).  No collectives.

Host-side mask compaction (as baseline): masked positions are dropped on the
host; keys padded to SPAD=640 (5 partition chunks), queries padded to QP=544
(max unmasked count is 543 for this data).  The kernel returns the softmax
NUMERATOR rows (64 per head) plus the DENOMINATOR row per head; the division
happens on the host during scatter.  This removes the on-device reciprocal
(Ln/Exp activations thrashed the ACT table loader: 33 table swaps = 42us) and
the DRAM-bounce broadcasts.

Engine budget per batch (target):
  PE: K proj (fp16, 2x320 chunks), V proj (bf16), scores (fp16 k weights ->
      single-pass FWL ldweights; 2x272 bank-aligned PSUM chunks), PV (bf16).
  ACT: only Exp, 40 calls of [128,544] (shift -45, softmax-shift-invariant).
  DVE: K-bias add, v_pv assembly, PV->SBUF copies.

Pipelining: per head, score matmul of chunk i+1 is emitted BEFORE the PV of
chunk i, so the PE never sits waiting on the exp; projections of batch b+1
are interleaved into the attention stream of batch b (PE never idles ->
HAM clock stays warm at 2.4 GHz).  PSUM: 2 pools x 2 bufs x 2 banks = 8.

The V bias is added via a rank-1 matmul whose lhs is the VALID-FLAG row
(instead of ones), so padded key rows stay exactly zero in v and contribute
nothing to the numerator; a 65th flag column gives the denominator.
"""

import numpy as np

BS, DA, SL, H = 16, 512, 1024, 8
N_CORES = 8
B = BS // N_CORES  # batches per core
P = 128
NT = DA // P       # channel tiles (4)
DH = DA // H       # head dim (64)

SPAD = 640         # padded compact key length (5 chunks of 128)
NSP = SPAD // P
QP = 544           # padded compact query length (max unmasked = 543)
CH = QP // 2       # 272: per-PSUM-bank matmul chunk (>=256 -> fp32r 1 cyc/col)
KCH = SPAD // 2    # 320: k-proj free-dim chunk

_CACHE: dict = {}


def build_nc(n_batches=B):
    from contextlib import ExitStack

    import concourse.bass as bass  # noqa: F401
    import concourse.tile as tile
    from concourse import bacc, mybir

    dt = mybir.dt.float32
    dtr = mybir.dt.float16
    bf16 = mybir.dt.bfloat16
    Act = mybir.ActivationFunctionType

    nc = bacc.Bacc("TRN2", target_bir_lowering=False, debug=False)

    q_d = nc.dram_tensor("q", [n_batches, DA, QP], dtr, kind="ExternalInput")
    kin_d = nc.dram_tensor("k_in", [n_batches, DA, SPAD], dtr, kind="ExternalInput")
    vin_d = nc.dram_tensor("v_in", [n_batches, DA, SPAD], bf16, kind="ExternalInput")
    kwT_d = nc.dram_tensor("k_wT", [DA, DA], dtr, kind="ExternalInput")
    vwT_d = nc.dram_tensor("v_wT", [DA, DA], bf16, kind="ExternalInput")
    kb_d = nc.dram_tensor("k_b", [DA], dt, kind="ExternalInput")
    vb_d = nc.dram_tensor("v_b", [DA], bf16, kind="ExternalInput")
    mrow_d = nc.dram_tensor("maskrow", [n_batches, SPAD], bf16, kind="ExternalInput")
    mcol_d = nc.dram_tensor("maskcol", [n_batches, SPAD], dt, kind="ExternalInput")
    out_d = nc.dram_tensor(
        "out", [n_batches, H, DH + 1, QP], bf16, kind="ExternalOutput"
    )

    with tile.TileContext(nc) as tc:
        with ExitStack() as ctx:
            consts = ctx.enter_context(tc.tile_pool(name="consts", bufs=1))
            qpool = ctx.enter_context(tc.tile_pool(name="qpool", bufs=2))
            kvpool = ctx.enter_context(tc.tile_pool(name="kvpool", bufs=2))
            mpool = ctx.enter_context(tc.tile_pool(name="mpool", bufs=2))
            kspool = ctx.enter_context(tc.tile_pool(name="kspool", bufs=2))
            vpvpool = ctx.enter_context(tc.tile_pool(name="vpvpool", bufs=2))
            epool = ctx.enter_context(tc.tile_pool(name="epool", bufs=3))
            opool = ctx.enter_context(tc.tile_pool(name="opool", bufs=4))
            psc = ctx.enter_context(tc.tile_pool(name="psc", bufs=3, space="PSUM"))
            ppv = ctx.enter_context(tc.tile_pool(name="ppv", bufs=1, space="PSUM"))

            # ---- constants ----
            kwT_sb = consts.tile([P, NT, DA], dtr)  # [p, ci, o]; c = ci*128+p
            for h_ in range(2):
                nc.sync.dma_start(
                    out=kwT_sb[:, 2 * h_ : 2 * h_ + 2],
                    in_=kwT_d.ap().rearrange("(ci p) o -> p ci o", p=P)[
                        :, 2 * h_ : 2 * h_ + 2
                    ],
                )
            vwT_sb = consts.tile([P, NT, DA], bf16)
            kb_col = consts.tile([P, NT], dt)  # k_b[o]; o = t*128+p
            nc.sync.dma_start(
                out=kb_col[:], in_=kb_d.ap().rearrange("(t p) -> p t", p=P)
            )
            vb_row = consts.tile([1, DA], bf16)
            nc.sync.dma_start(
                out=vb_row[:], in_=vb_d.ap().rearrange("(a o) -> a o", a=1)
            )
            ones8 = consts.tile([P, H], dt)
            nc.vector.memset(ones8[:], 1.0)
            negC = consts.tile([P, 1], dt)
            nc.vector.memset(negC[:], -45.0)

            def emit_load(b):
                # batch 0: k-side loads ride the (idle) ACT queue so kproj
                # can start while the sync queue streams the v/q side.
                kin = kvpool.tile([P, NT, SPAD], dtr, name=f"kin{b}", tag="kin")
                for ci in range(NT):
                    nc.sync.dma_start(
                        out=kin[:, ci, :],
                        in_=kin_d.ap()[b].rearrange("(t p) s -> p t s", p=P)[:, ci],
                    )
                vin = kvpool.tile([P, NT, SPAD], bf16, name=f"vin{b}", tag="vin")
                for ci in range(NT):
                    nc.sync.dma_start(
                        out=vin[:, ci, :],
                        in_=vin_d.ap()[b].rearrange("(t p) s -> p t s", p=P)[:, ci],
                    )
                q_sb = qpool.tile([P, NT, QP], dtr, name=f"q{b}", tag="q")
                for ci in range(NT):
                    nc.sync.dma_start(
                        out=q_sb[:, ci, :],
                        in_=q_d.ap()[b].rearrange("(t p) s -> p t s", p=P)[:, ci],
                    )
                mrow = mpool.tile([1, SPAD], bf16, name=f"mrow{b}", tag="mrow")
                nc.sync.dma_start(
                    out=mrow[:], in_=mrow_d.ap()[b].rearrange("(a s) -> a s", a=1)
                )
                mcol = mpool.tile([P, NSP], dt, name=f"mcol{b}", tag="mcol")
                nc.sync.dma_start(
                    out=mcol[:], in_=mcol_d.ap()[b].rearrange("(i p) -> p i", p=P)
                )
                k_sb = kspool.tile([P, NT, SPAD], dtr, name=f"k{b}", tag="k")
                v_pv = vpvpool.tile(
                    [P, NSP, H, DH + 1], bf16, name=f"vpv{b}", tag="vpv"
                )
                return dict(b=b, q=q_sb, kin=kin, vin=vin, mrow=mrow, mcol=mcol,
                            k=k_sb, vpv=v_pv)

            def emit_proj(S, g):
                b = S["b"]
                if g < NT:  # K projection tile t=g
                    t = g
                    kp = psc.tile([P, 2, 512], dt, name=f"kp{b}_{t}", tag="ps")
                    for j in range(2):
                        for ci in range(NT):
                            nc.tensor.matmul(
                                kp[:, j, 0:KCH],
                                kwT_sb[:, ci, t * P : (t + 1) * P],
                                S["kin"][:, ci, j * KCH : (j + 1) * KCH],
                                start=(ci == 0),
                                stop=(ci == NT - 1),
                            )
                    nc.vector.tensor_scalar_add(
                        S["k"][:, t, :].rearrange("p (j s) -> p j s", j=2),
                        kp[:, :, 0:KCH],
                        kb_col[:, t : t + 1],
                    )
                else:  # V projection chunk i=g-NT
                    i = g - NT
                    vp = psc.tile([P, 2, 512], dt, name=f"vp{b}_{i}", tag="ps")
                    for ci in range(NT):
                        nc.tensor.matmul(
                            vp[:, 0, :],
                            S["vin"][:, ci, i * P : (i + 1) * P],
                            vwT_sb[:, ci, :],
                            start=(ci == 0),
                            stop=False,
                        )
                    # bias via rank-1 (valid-flag row x v_b): padded rows stay 0
                    nc.tensor.matmul(
                        vp[:, 0, :],
                        S["mrow"][0:1, i * P : (i + 1) * P],
                        vb_row[0:1, :],
                        start=False,
                        stop=True,
                    )
                    nc.vector.tensor_copy(
                        S["vpv"][:, i, :, 0:DH],
                        vp[:, 0, :].rearrange("p (h d) -> p h d", h=H),
                    )
                    nc.vector.tensor_scalar_mul(
                        S["vpv"][:, i, :, DH], ones8[:, 0:H], S["mcol"][:, i : i + 1]
                    )

            def emit_exp(S, h, i, sc):
                b = S["b"]
                es = epool.tile([P, QP], bf16, name=f"es{b}_{h}_{i}", tag="es")
                nc.scalar.activation(
                    es[:, :].rearrange("p (j c) -> p j c", j=2),
                    sc[:, :, 0:CH],
                    Act.Exp,
                    bias=negC[:, 0:1],
                )
                return es

            def emit_head(S, h, fill=()):
                fill = list(fill)
                b = S["b"]
                t, base = h // 2, (h % 2) * DH
                pv = ppv.tile([P, 2, 512], dt, name=f"pv{b}_{h}", tag="pv")

                def sc_mm(sc, j, i):
                    nc.tensor.matmul(
                        sc[:, j, 0:CH],
                        S["k"][base : base + DH, t, i * P : (i + 1) * P],
                        S["q"][base : base + DH, t, j * CH : (j + 1) * CH],
                        start=True,
                        stop=True,
                    )

                def pv_mm(es, j, i):
                    nc.tensor.matmul(
                        pv[0 : DH + 1, j, 0:CH],
                        S["vpv"][:, i, h, :],
                        es[:, j * CH : (j + 1) * CH],
                        start=(i == 0),
                        stop=(i == NSP - 1),
                    )

                # software pipeline: sc(i) chunks interleave with pv(i-1)
                # chunks so consecutive PE instructions never reload the
                # same weights (the reload would not hide behind the mm).
                ess = {}
                scs = {}
                for i in range(NSP):
                    if fill:
                        fill.pop(0)()
                    sc = psc.tile([P, 2, 512], dt, name=f"sc{b}_{h}_{i}", tag="ps")
                    if i == 0:
                        sc_mm(sc, 0, i)
                        sc_mm(sc, 1, i)
                    else:
                        ep = ess[i - 1]
                        sc_mm(sc, 0, i)
                        pv_mm(ep, 0, i - 1)
                        sc_mm(sc, 1, i)
                        pv_mm(ep, 1, i - 1)
                    scs[i] = sc
                    ess[i] = emit_exp(S, h, i, sc)
                ep = ess[NSP - 1]
                pv_mm(ep, 0, NSP - 1)
                pv_mm(ep, 1, NSP - 1)
                o_raw = opool.tile([DH + 1, QP], bf16, name=f"o{b}_{h}", tag="o")
                nc.vector.tensor_copy(
                    o_raw[:, :].rearrange("p (j c) -> p j c", j=2),
                    pv[0 : DH + 1, :, 0:CH],
                )
                nc.sync.dma_start(out=out_d.ap()[b, h], in_=o_raw[:, :])

            # projections of batch b+1 interleave into attention of batch b
            # (late slots: the b+1 DMAs must have landed)
            PROJ_SLOTS = {2: [0], 3: [1], 4: [2], 5: [3]}

            states = [None] * n_batches
            states[0] = emit_load(0)
            # v-weights stream after the k-side inputs (vproj needs them
            # only once head-0 attention starts)
            nc.sync.dma_start(
                out=vwT_sb[:], in_=vwT_d.ap().rearrange("(ci p) o -> p ci o", p=P)
            )
            for g in range(NT):
                emit_proj(states[0], g)
            for b in range(n_batches):
                S = states[b]
                if b + 1 < n_batches:
                    states[b + 1] = emit_load(b + 1)
                for h in range(H):
                    fill = ()
                    if h == 0:
                        fill = [
                            (lambda g=g: emit_proj(S, NT + g)) for g in range(NSP)
                        ]
                    if b + 1 < n_batches:
                        for g in PROJ_SLOTS.get(h, []):
                            emit_proj(states[b + 1], g)
                    emit_head(S, h, fill=fill)

    nc.compile()
    return nc


def _get_nc():
    if "nc" not in _CACHE:
        _CACHE["nc"] = build_nc()
    return _CACHE["nc"]


def _prepare(inputs):
    """Host-side compaction + sharding.  Returns (in_maps, keeps list)."""
    import ml_dtypes

    bf = ml_dtypes.bfloat16
    f16 = np.float16
    q = np.asarray(inputs["q"], dtype=np.float32)
    k_in = np.asarray(inputs["k_in"], dtype=np.float32)
    v_in = np.asarray(inputs["v_in"], dtype=np.float32)
    k_w = np.asarray(inputs["k_w"], dtype=np.float32)
    k_b = np.asarray(inputs["k_b"], dtype=np.float32)
    v_w = np.asarray(inputs["v_w"], dtype=np.float32)
    v_b = np.asarray(inputs["v_b"], dtype=np.float32)
    gamma = np.asarray(inputs["gbn_gamma"], dtype=np.float32)
    gs = np.asarray(inputs["gbn_s"], dtype=np.float32)
    mask = np.asarray(inputs["mask"]).reshape(BS, SL)

    # GBN affine: only gamma/sd matters (additive part is softmax-shift-
    # invariant); fold into q per head.
    a = (gamma / gs).astype(np.float32)
    q_scaled = (
        (q.reshape(BS, H, DH, SL) * a[None, :, None, None]).reshape(BS, DA, SL)
    ).astype(np.float32)

    keeps = [np.flatnonzero(mask[b] == 0) for b in range(BS)]
    for b, kidx in enumerate(keeps):
        if len(kidx) > QP:
            raise ValueError(f"batch {b}: {len(kidx)} unmasked > QP={QP}")

    qc = np.zeros((BS, DA, QP), f16)
    kc = np.zeros((BS, DA, SPAD), f16)
    vc = np.zeros((BS, DA, SPAD), bf)
    mrow = np.zeros((BS, SPAD), bf)
    mcol = np.zeros((BS, SPAD), np.float32)
    for b, kidx in enumerate(keeps):
        n = len(kidx)
        qc[b, :, :n] = q_scaled[b][:, kidx].astype(f16)
        kc[b, :, :n] = k_in[b][:, kidx].astype(f16)
        vc[b, :, :n] = v_in[b][:, kidx].astype(bf)
        mrow[b, :n] = 1.0
        mcol[b, :n] = 1.0

    k_wT = np.ascontiguousarray(k_w.T, dtype=f16)
    v_wT = np.ascontiguousarray(v_w.T).astype(bf)

    in_maps = []
    for c in range(N_CORES):
        sl = slice(c * B, (c + 1) * B)
        in_maps.append(
            {
                "q": np.ascontiguousarray(qc[sl]),
                "k_in": np.ascontiguousarray(kc[sl]),
                "v_in": np.ascontiguousarray(vc[sl]),
                "k_wT": k_wT,
                "v_wT": v_wT,
                "k_b": k_b,
                "v_b": v_b.astype(bf),
                "maskrow": np.ascontiguousarray(mrow[sl]),
                "maskcol": np.ascontiguousarray(mcol[sl]),
            }
        )
    return in_maps, keeps


def _scatter(results, keeps) -> np.ndarray:
    """Divide numerator rows by the denominator row and scatter back."""
    out = np.zeros((BS, DA, SL), np.float32)
    for c in range(N_CORES):
        oc = np.asarray(results[c]["out"], dtype=np.float32)  # [B, H, 65, QP]
        for bb in range(B):
            b = c * B + bb
            kidx = keeps[b]
            n = len(kidx)
            num = oc[bb, :, 0:DH, :n]          # [H, 64, n]
            den = oc[bb, :, DH, :n]            # [H, n]
            den = np.where(den == 0.0, 1.0, den)
            out[b][:, kidx] = (num / den[:, None, :]).reshape(DA, n)
    return out


def kernel(**inputs) -> np.ndarray:
    from concourse.bass_utils import run_bass_kernel_spmd

    in_maps, keeps = _prepare(inputs)
    nc = _get_nc()
    res = run_bass_kernel_spmd(nc, in_maps, list(range(N_CORES)))
    return _scatter(res.results, keeps)


# revision 16
# speedup vs baseline: 1.2556x; 1.2556x over previous
"""Trainium2 Bass kernel: multi-head attention with 1x1-conv K/V projections,
per-head GhostBatchNorm (eval-mode affine), key+query masking, softmax.

Sharding: pure data parallelism over the batch axis (16 batches -> 8 cores,
2 per core).  No collectives.

Host-side mask compaction (as baseline): masked positions are dropped on the
host; keys padded to SPAD=640 (5 partition chunks), queries padded to QP=544
(max unmasked count is 543 for this data).  The kernel returns the softmax
NUMERATOR rows (64 per head) plus the DENOMINATOR row per head; the division
happens on the host during scatter.  This removes the on-device reciprocal
(Ln/Exp activations thrashed the ACT table loader: 33 table swaps = 42us) and
the DRAM-bounce broadcasts.

Engine budget per batch (target):
  PE: K proj (fp16, 2x320 chunks), V proj (bf16), scores (fp16 k weights ->
      single-pass FWL ldweights; 2x272 bank-aligned PSUM chunks), PV (bf16).
  ACT: only Exp, 40 calls of [128,544] (shift -45, softmax-shift-invariant).
  DVE: K-bias add, v_pv assembly, PV->SBUF copies.

Pipelining: per head, score matmul of chunk i+1 is emitted BEFORE the PV of
chunk i, so the PE never sits waiting on the exp; projections of batch b+1
are interleaved into the attention stream of batch b (PE never idles ->
HAM clock stays warm at 2.4 GHz).  PSUM: 2 pools x 2 bufs x 2 banks = 8.

The V bias is added via a rank-1 matmul whose lhs is the VALID-FLAG row
(instead of ones), so padded key rows stay exactly zero in v and contribute
nothing to the numerator; a 65th flag column gives the denominator.
"""

import numpy as np

BS, DA, SL, H = 16, 512, 1024, 8
N_CORES = 8
B = BS // N_CORES  # batches per core
P = 128
NT = DA // P       # channel tiles (4)
DH = DA // H       # head dim (64)

SPAD = 640         # padded compact key length (5 chunks of 128)
NSP = SPAD // P
QP = 544           # padded compact query length (max unmasked = 543)
CH = QP // 2       # 272: per-PSUM-bank matmul chunk (>=256 -> fp32r 1 cyc/col)
KCH = SPAD // 2    # 320: k-proj free-dim chunk

_CACHE: dict = {}


def build_nc(n_batches=B):
    from contextlib import ExitStack

    import concourse.bass as bass  # noqa: F401
    import concourse.tile as tile
    from concourse import bacc, mybir

    dt = mybir.dt.float32
    dtr = mybir.dt.float16
    bf16 = mybir.dt.bfloat16
    Act = mybir.ActivationFunctionType

    nc = bacc.Bacc("TRN2", target_bir_lowering=False, debug=False)

    q_d = nc.dram_tensor("q", [n_batches, DA, QP], dtr, kind="ExternalInput")
    kin_d = nc.dram_tensor("k_in", [n_batches, DA, SPAD], dtr, kind="ExternalInput")
    vin_d = nc.dram_tensor("v_in", [n_batches, DA, SPAD], bf16, kind="ExternalInput")
    kwT_d = nc.dram_tensor("k_wT", [DA, DA], dtr, kind="ExternalInput")
    vwT_d = nc.dram_tensor("v_wT", [DA, DA], bf16, kind="ExternalInput")
    kb_d = nc.dram_tensor("k_b", [DA], dt, kind="ExternalInput")
    vb_d = nc.dram_tensor("v_b", [DA], bf16, kind="ExternalInput")
    mrow_d = nc.dram_tensor("maskrow", [n_batches, SPAD], bf16, kind="ExternalInput")
    mcol_d = nc.dram_tensor("maskcol", [n_batches, SPAD], dt, kind="ExternalInput")
    out_d = nc.dram_tensor(
        "out", [n_batches, H, DH + 1, QP], bf16, kind="ExternalOutput"
    )

    with tile.TileContext(nc) as tc:
        with ExitStack() as ctx:
            consts = ctx.enter_context(tc.tile_pool(name="consts", bufs=1))
            qpool = ctx.enter_context(tc.tile_pool(name="qpool", bufs=2))
            kvpool = ctx.enter_context(tc.tile_pool(name="kvpool", bufs=2))
            mpool = ctx.enter_context(tc.tile_pool(name="mpool", bufs=2))
            kspool = ctx.enter_context(tc.tile_pool(name="kspool", bufs=2))
            vpvpool = ctx.enter_context(tc.tile_pool(name="vpvpool", bufs=2))
            epool = ctx.enter_context(tc.tile_pool(name="epool", bufs=3))
            opool = ctx.enter_context(tc.tile_pool(name="opool", bufs=4))
            psc = ctx.enter_context(tc.tile_pool(name="psc", bufs=3, space="PSUM"))
            ppv = ctx.enter_context(tc.tile_pool(name="ppv", bufs=1, space="PSUM"))

            # ---- constants ----
            kwT_sb = consts.tile([P, NT, DA], dtr)  # [p, ci, o]; c = ci*128+p
            nc.sync.dma_start(
                out=kwT_sb[:], in_=kwT_d.ap().rearrange("(ci p) o -> p ci o", p=P)
            )
            vwT_sb = consts.tile([P, NT, DA], bf16)
            nc.sync.dma_start(
                out=vwT_sb[:], in_=vwT_d.ap().rearrange("(ci p) o -> p ci o", p=P)
            )
            kb_col = consts.tile([P, NT], dt)  # k_b[o]; o = t*128+p
            nc.sync.dma_start(
                out=kb_col[:], in_=kb_d.ap().rearrange("(t p) -> p t", p=P)
            )
            vb_row = consts.tile([1, DA], bf16)
            nc.sync.dma_start(
                out=vb_row[:], in_=vb_d.ap().rearrange("(a o) -> a o", a=1)
            )
            ones8 = consts.tile([P, H], dt)
            nc.vector.memset(ones8[:], 1.0)
            negC = consts.tile([P, 1], dt)
            nc.vector.memset(negC[:], -45.0)

            def emit_load(b):
                # batch 0: k-side loads ride the (idle) ACT queue so kproj
                # can start while the sync queue streams the v/q side.
                kin = kvpool.tile([P, NT, SPAD], dtr, name=f"kin{b}", tag="kin")
                for ci in range(NT):
                    nc.sync.dma_start(
                        out=kin[:, ci, :],
                        in_=kin_d.ap()[b].rearrange("(t p) s -> p t s", p=P)[:, ci],
                    )
                vin = kvpool.tile([P, NT, SPAD], bf16, name=f"vin{b}", tag="vin")
                for ci in range(NT):
                    nc.sync.dma_start(
                        out=vin[:, ci, :],
                        in_=vin_d.ap()[b].rearrange("(t p) s -> p t s", p=P)[:, ci],
                    )
                q_sb = qpool.tile([P, NT, QP], dtr, name=f"q{b}", tag="q")
                for ci in range(NT):
                    nc.sync.dma_start(
                        out=q_sb[:, ci, :],
                        in_=q_d.ap()[b].rearrange("(t p) s -> p t s", p=P)[:, ci],
                    )
                mrow = mpool.tile([1, SPAD], bf16, name=f"mrow{b}", tag="mrow")
                nc.sync.dma_start(
                    out=mrow[:], in_=mrow_d.ap()[b].rearrange("(a s) -> a s", a=1)
                )
                mcol = mpool.tile([P, NSP], dt, name=f"mcol{b}", tag="mcol")
                nc.sync.dma_start(
                    out=mcol[:], in_=mcol_d.ap()[b].rearrange("(i p) -> p i", p=P)
                )
                k_sb = kspool.tile([P, NT, SPAD], dtr, name=f"k{b}", tag="k")
                v_pv = vpvpool.tile(
                    [P, NSP, H, DH + 1], bf16, name=f"vpv{b}", tag="vpv"
                )
                return dict(b=b, q=q_sb, kin=kin, vin=vin, mrow=mrow, mcol=mcol,
                            k=k_sb, vpv=v_pv)

            def emit_proj(S, g):
                b = S["b"]
                if g < NT:  # K projection tile t=g
                    t = g
                    kp = psc.tile([P, 2, 512], dt, name=f"kp{b}_{t}", tag="ps")
                    for j in range(2):
                        for ci in range(NT):
                            nc.tensor.matmul(
                                kp[:, j, 0:KCH],
                                kwT_sb[:, ci, t * P : (t + 1) * P],
                                S["kin"][:, ci, j * KCH : (j + 1) * KCH],
                                start=(ci == 0),
                                stop=(ci == NT - 1),
                            )
                    nc.vector.tensor_scalar_add(
                        S["k"][:, t, :].rearrange("p (j s) -> p j s", j=2),
                        kp[:, :, 0:KCH],
                        kb_col[:, t : t + 1],
                    )
                else:  # V projection chunk i=g-NT
                    i = g - NT
                    vp = psc.tile([P, 2, 512], dt, name=f"vp{b}_{i}", tag="ps")
                    for ci in range(NT):
                        nc.tensor.matmul(
                            vp[:, 0, :],
                            S["vin"][:, ci, i * P : (i + 1) * P],
                            vwT_sb[:, ci, :],
                            start=(ci == 0),
                            stop=False,
                        )
                    # bias via rank-1 (valid-flag row x v_b): padded rows stay 0
                    nc.tensor.matmul(
                        vp[:, 0, :],
                        S["mrow"][0:1, i * P : (i + 1) * P],
                        vb_row[0:1, :],
                        start=False,
                        stop=True,
                    )
                    nc.vector.tensor_copy(
                        S["vpv"][:, i, :, 0:DH],
                        vp[:, 0, :].rearrange("p (h d) -> p h d", h=H),
                    )
                    nc.vector.tensor_scalar_mul(
                        S["vpv"][:, i, :, DH], ones8[:, 0:H], S["mcol"][:, i : i + 1]
                    )

            def emit_exp(S, h, i, sc):
                b = S["b"]
                es = epool.tile([P, QP], bf16, name=f"es{b}_{h}_{i}", tag="es")
                nc.scalar.activation(
                    es[:, :].rearrange("p (j c) -> p j c", j=2),
                    sc[:, :, 0:CH],
                    Act.Exp,
                    bias=negC[:, 0:1],
                )
                return es

            def emit_head(S, h, fill=()):
                fill = list(fill)
                b = S["b"]
                t, base = h // 2, (h % 2) * DH
                pv = ppv.tile([P, 2, 512], dt, name=f"pv{b}_{h}", tag="pv")

                def sc_mm(sc, j, i):
                    nc.tensor.matmul(
                        sc[:, j, 0:CH],
                        S["k"][base : base + DH, t, i * P : (i + 1) * P],
                        S["q"][base : base + DH, t, j * CH : (j + 1) * CH],
                        start=True,
                        stop=True,
                    )

                def pv_mm(es, j, i):
                    nc.tensor.matmul(
                        pv[0 : DH + 1, j, 0:CH],
                        S["vpv"][:, i, h, :],
                        es[:, j * CH : (j + 1) * CH],
                        start=(i == 0),
                        stop=(i == NSP - 1),
                    )

                # software pipeline: sc(i) chunks interleave with pv(i-1)
                # chunks so consecutive PE instructions never reload the
                # same weights (the reload would not hide behind the mm).
                ess = {}
                scs = {}
                for i in range(NSP):
                    if fill:
                        fill.pop(0)()
                    sc = psc.tile([P, 2, 512], dt, name=f"sc{b}_{h}_{i}", tag="ps")
                    if i == 0:
                        sc_mm(sc, 0, i)
                        sc_mm(sc, 1, i)
                    else:
                        ep = ess[i - 1]
                        sc_mm(sc, 0, i)
                        pv_mm(ep, 0, i - 1)
                        sc_mm(sc, 1, i)
                        pv_mm(ep, 1, i - 1)
                    scs[i] = sc
                    ess[i] = emit_exp(S, h, i, sc)
                ep = ess[NSP - 1]
                pv_mm(ep, 0, NSP - 1)
                pv_mm(ep, 1, NSP - 1)
                o_raw = opool.tile([DH + 1, QP], bf16, name=f"o{b}_{h}", tag="o")
                nc.vector.tensor_copy(
                    o_raw[:, :].rearrange("p (j c) -> p j c", j=2),
                    pv[0 : DH + 1, :, 0:CH],
                )
                nc.sync.dma_start(out=out_d.ap()[b, h], in_=o_raw[:, :])

            # projections of batch b+1 interleave into attention of batch b
            # (late slots: the b+1 DMAs must have landed)
            PROJ_SLOTS = {1: [0], 2: [1], 3: [2], 4: [3], 5: [4], 6: [5, 6], 7: [7, 8]}

            states = [None] * n_batches
            states[0] = emit_load(0)
            for g in range(NT):
                emit_proj(states[0], g)
            for b in range(n_batches):
                S = states[b]
                if b + 1 < n_batches:
                    states[b + 1] = emit_load(b + 1)
                for h in range(H):
                    fill = ()
                    if b == 0 and h == 0:
                        fill = [
                            (lambda g=g: emit_proj(S, NT + g)) for g in range(NSP)
                        ]
                    if b + 1 < n_batches:
                        for g in PROJ_SLOTS.get(h, []):
                            emit_proj(states[b + 1], g)
                    emit_head(S, h, fill=fill)

    nc.compile()
    return nc


def _get_nc():
    if "nc" not in _CACHE:
        _CACHE["nc"] = build_nc()
    return _CACHE["nc"]


def _prepare(inputs):
    """Host-side compaction + sharding.  Returns (in_maps, keeps list)."""
    import ml_dtypes

    bf = ml_dtypes.bfloat16
    f16 = np.float16
    q = np.asarray(inputs["q"], dtype=np.float32)
    k_in = np.asarray(inputs["k_in"], dtype=np.float32)
    v_in = np.asarray(inputs["v_in"], dtype=np.float32)
    k_w = np.asarray(inputs["k_w"], dtype=np.float32)
    k_b = np.asarray(inputs["k_b"], dtype=np.float32)
    v_w = np.asarray(inputs["v_w"], dtype=np.float32)
    v_b = np.asarray(inputs["v_b"], dtype=np.float32)
    gamma = np.asarray(inputs["gbn_gamma"], dtype=np.float32)
    gs = np.asarray(inputs["gbn_s"], dtype=np.float32)
    mask = np.asarray(inputs["mask"]).reshape(BS, SL)

    # GBN affine: only gamma/sd matters (additive part is softmax-shift-
    # invariant); fold into q per head.
    a = (gamma / gs).astype(np.float32)
    q_scaled = (
        (q.reshape(BS, H, DH, SL) * a[None, :, None, None]).reshape(BS, DA, SL)
    ).astype(np.float32)

    keeps = [np.flatnonzero(mask[b] == 0) for b in range(BS)]
    for b, kidx in enumerate(keeps):
        if len(kidx) > QP:
            raise ValueError(f"batch {b}: {len(kidx)} unmasked > QP={QP}")

    qc = np.zeros((BS, DA, QP), f16)
    kc = np.zeros((BS, DA, SPAD), f16)
    vc = np.zeros((BS, DA, SPAD), bf)
    mrow = np.zeros((BS, SPAD), bf)
    mcol = np.zeros((BS, SPAD), np.float32)
    for b, kidx in enumerate(keeps):
        n = len(kidx)
        qc[b, :, :n] = q_scaled[b][:, kidx].astype(f16)
        kc[b, :, :n] = k_in[b][:, kidx].astype(f16)
        vc[b, :, :n] = v_in[b][:, kidx].astype(bf)
        mrow[b, :n] = 1.0
        mcol[b, :n] = 1.0

    k_wT = np.ascontiguousarray(k_w.T, dtype=f16)
    v_wT = np.ascontiguousarray(v_w.T).astype(bf)

    in_maps = []
    for c in range(N_CORES):
        sl = slice(c * B, (c + 1) * B)
        in_maps.append(
            {
                "q": np.ascontiguousarray(qc[sl]),
                "k_in": np.ascontiguousarray(kc[sl]),
                "v_in": np.ascontiguousarray(vc[sl]),
                "k_wT": k_wT,
                "v_wT": v_wT,
                "k_b": k_b,
                "v_b": v_b.astype(bf),
                "maskrow": np.ascontiguousarray(mrow[sl]),
                "maskcol": np.ascontiguousarray(mcol[sl]),
            }
        )
    return in_maps, keeps


def _scatter(results, keeps) -> np.ndarray:
    """Divide numerator rows by the denominator row and scatter back."""
    out = np.zeros((BS, DA, SL), np.float32)
    for c in range(N_CORES):
        oc = np.asarray(results[c]["out"], dtype=np.float32)  # [B, H, 65, QP]
        for bb in range(B):
            b = c * B + bb
            kidx = keeps[b]
            n = len(kidx)
            num = oc[bb, :, 0:DH, :n]          # [H, 64, n]
            den = oc[bb, :, DH, :n]            # [H, n]
            den = np.where(den == 0.0, 1.0, den)
            out[b][:, kidx] = (num / den[:, None, :]).reshape(DA, n)
    return out


def kernel(**inputs) -> np.ndarray:
    from concourse.bass_utils import run_bass_kernel_spmd

    in_maps, keeps = _prepare(inputs)
    nc = _get_nc()
    res = run_bass_kernel_spmd(nc, in_maps, list(range(N_CORES)))
    return _scatter(res.results, keeps)
